# revision 18
# baseline (speedup 1.0000x reference)
"""Trainium2 Bass kernel for BatchAll triplet loss.

Reference computation (B=512, D=1024):
    pw = img @ sent.T                                  [B, B]
    t[a,p,n] = pw[a,p] - pw[a,n] + margin
    valid[a,p,n] = (lab[a]==lab[p]) & (lab[a]!=lab[n])
    loss = sum(relu(valid*t)) / (count(valid*t > EPS) + EPS)

Strategy: the batch is class-sorted on the host (a pure permutation of the
(image, sentence, label) triples; the loss is permutation invariant), then
anchors are sharded across 8 cores (64 each, C = core*64). After sorting,
the positives of anchor g all live in a contiguous class run within
(g-16, g+16) as long as every class has <= 16 members (checked on host;
dense fallback otherwise). So for anchor a (local), the p-axis can be
restricted to a 64-wide, 32-aligned window inside the core's 128-wide
sentence window [C-32, C+96).

Per core:
    pw rows over full n [64, 512] and over the window [64, 128] (PE).
    w[a,pwin] = pw+margin if same label else -1e30      [64, 128]
    z[a,n]    = -pw       if label differs else -1e30   [64, 512] bf16
    wT = transpose(w)  -> per-anchor bias columns       [128, 64]
    Main loop packs TWO anchors per tile: partitions = 2 x 64-window,
    free = all 512 n.  PE broadcasts the two z rows via a two-hot
    selector matmul; ACT applies relu(z + w) with the stacked window
    bias and accum_out row-sums; DVE counts r > EPS with accum_out.
Host combines the 8 (sum, count) pairs and divides.
"""

import numpy as np
from contextlib import ExitStack

B = 512
D = 1024
NCORES = 8
A = B // NCORES   # 64 anchors per core
KT = D // 128     # 8 contraction tiles
NT = B // 128     # 4 n-tiles per anchor (dense variant)
W = 128           # per-core sentence window width
MARGIN = 0.2
EPS = 1e-16
BIG = 1e30
MAXC_WIN = 16     # windowed variant valid iff max class size <= this

_CACHE = {}


def _lo_local(a):
    """32-aligned offset of anchor a's 64-wide window inside the core's
    128-wide window (anchor a sits at local window position 32 + a)."""
    return 32 * ((a - 15) // 32) + 32


def _build_win():
    """Class-sorted windowed kernel (primary)."""
    import concourse.mybir as mybir
    import concourse.tile as tile
    from concourse import bacc
    from concourse.masks import make_identity

    f32 = mybir.dt.float32
    bf16 = mybir.dt.bfloat16
    Alu = mybir.AluOpType
    Act = mybir.ActivationFunctionType
    Ax = mybir.AxisListType

    nc = bacc.Bacc("TRN2", target_bir_lowering=False, debug=False,
                   num_devices=NCORES)

    imgT_d = nc.dram_tensor("imgT", [D, A], bf16, kind="ExternalInput")
    sentT_d = nc.dram_tensor("sentT", [D, B], bf16, kind="ExternalInput")
    sentWinT_d = nc.dram_tensor("sentWinT", [D, W], bf16, kind="ExternalInput")
    labf_d = nc.dram_tensor("labf", [B], f32, kind="ExternalInput")
    labw_d = nc.dram_tensor("labw", [W], f32, kind="ExternalInput")
    labc_d = nc.dram_tensor("labc", [A], f32, kind="ExternalInput")
    out_d = nc.dram_tensor("out", [2], f32, kind="ExternalOutput")

    NP = A // 2  # anchor pairs per core

    with tile.TileContext(nc) as tc:
        with ExitStack() as ctx:
            singles = ctx.enter_context(tc.tile_pool(name="singles", bufs=1))
            rpool = ctx.enter_context(tc.tile_pool(name="rpool", bufs=6))
            mpool = ctx.enter_context(tc.tile_pool(name="mpool", bufs=6))
            spsum = ctx.enter_context(
                tc.tile_pool(name="spsum", bufs=1, space="PSUM"))
            wpsum = ctx.enter_context(
                tc.tile_pool(name="wpsum", bufs=6, space="PSUM"))

            # ---- constants ----
            ones_r = singles.tile([1, A], f32)
            nc.vector.memset(ones_r, 1.0)
            ones_c = singles.tile([128, 1], f32)
            nc.vector.memset(ones_c, 1.0)
            ident = singles.tile([A, A], f32)
            make_identity(nc, ident)
            # sel_all[k, ar, h, v] = 1 iff k == 2*ar + h  (two-hot selector)
            sel_all = singles.tile([A, NP, 2, 64], bf16)
            nc.gpsimd.memset(sel_all, 0.0)
            nc.gpsimd.affine_select(
                out=sel_all, in_=sel_all, compare_op=mybir.AluOpType.not_equal,
                fill=1.0, base=0, pattern=[[-2, NP], [-1, 2], [0, 64]],
                channel_multiplier=1)

            # ---- loads ----
            imgT = singles.tile([128, KT, A], bf16)
            nc.sync.dma_start(
                out=imgT, in_=imgT_d.ap().rearrange("(t p) m -> p t m", p=128))
            sentT = singles.tile([128, KT, B], bf16)
            nc.sync.dma_start(
                out=sentT, in_=sentT_d.ap().rearrange("(t p) m -> p t m", p=128))
            sentWinT = singles.tile([128, KT, W], bf16)
            nc.sync.dma_start(
                out=sentWinT,
                in_=sentWinT_d.ap().rearrange("(t p) m -> p t m", p=128))
            lab_row = singles.tile([1, B], f32)
            nc.sync.dma_start(
                out=lab_row, in_=labf_d.ap().rearrange("(o b) -> o b", o=1))
            labw_row = singles.tile([1, W], f32)
            nc.sync.dma_start(
                out=labw_row, in_=labw_d.ap().rearrange("(o b) -> o b", o=1))
            labc_col = singles.tile([A, 1], f32)
            nc.sync.dma_start(
                out=labc_col, in_=labc_d.ap().rearrange("(a o) -> a o", o=1))

            # ---- pairwise rows (full n for z, window for w) ----
            pw_ps = spsum.tile([A, B], f32, tag="sA")
            pwin_ps = spsum.tile([A, W], f32, tag="sB")
            for kt in range(KT):
                nc.tensor.matmul(pw_ps, lhsT=imgT[:, kt, :], rhs=sentT[:, kt, :],
                                 start=(kt == 0), stop=(kt == KT - 1))
            for kt in range(KT):
                nc.tensor.matmul(pwin_ps, lhsT=imgT[:, kt, :],
                                 rhs=sentWinT[:, kt, :],
                                 start=(kt == 0), stop=(kt == KT - 1))
            w_win = singles.tile([A, W], f32)
            nc.vector.tensor_scalar(w_win, pwin_ps, MARGIN, None, Alu.add)

            # ---- full-n masks -> z rows (bf16) ----
            labB_ps = spsum.tile([A, B], f32, tag="sB")
            nc.tensor.matmul(labB_ps, lhsT=ones_r, rhs=lab_row)
            eqP = singles.tile([A, B], f32)
            nc.vector.tensor_scalar(eqP, labB_ps, labc_col, None, Alu.is_equal)
            negneq = singles.tile([A, B], f32)
            nc.vector.tensor_scalar(negneq, eqP, 1.0, -1.0, Alu.subtract,
                                    Alu.mult)
            penN = singles.tile([A, B], f32)
            nc.vector.tensor_scalar(penN, eqP, -BIG, None, Alu.mult)
            z1 = singles.tile([A, B], f32)
            nc.vector.tensor_scalar(z1, pw_ps, -1.0, None, Alu.mult)
            nc.vector.tensor_mul(z1, z1, negneq)
            z_bf = singles.tile([A, B], bf16)
            nc.vector.tensor_add(z_bf, z1, penN)

            # ---- window masks -> w rows ----
            labBw_ps = spsum.tile([A, W], f32, tag="sB")
            nc.tensor.matmul(labBw_ps, lhsT=ones_r, rhs=labw_row)
            eqW = singles.tile([A, W], f32)
            nc.vector.tensor_scalar(eqW, labBw_ps, labc_col, None, Alu.is_equal)
            penW = singles.tile([A, W], f32)
            nc.vector.tensor_scalar(penW, eqW, 1.0, BIG, Alu.subtract, Alu.mult)
            nc.vector.tensor_mul(w_win, w_win, eqW)
            nc.vector.tensor_add(w_win, w_win, penW)

            # ---- transpose w -> bias columns ----
            wT_ps = spsum.tile([W, A], f32, tag="sA")
            nc.tensor.transpose(wT_ps, w_win, ident)
            wT_sb = singles.tile([W, A], f32)
            nc.scalar.copy(wT_sb, wT_ps)

            # bias2[h*64+v, ar] = wT[lo(2ar+h)+v, 2ar+h]
            bias2 = singles.tile([128, NP], f32)
            wT_v = wT_sb.rearrange("p (ar h) -> p ar h", h=2)
            runs_even = [(0, 8, 0), (8, 24, 32), (24, 32, 64)]
            runs_odd = [(0, 7, 0), (7, 23, 32), (23, 32, 64)]
            for h, runs in ((0, runs_even), (1, runs_odd)):
                for r0, r1, lo in runs:
                    if lo % 64 == 0:
                        nc.scalar.copy(bias2[h * 64:(h + 1) * 64, r0:r1],
                                       wT_v[lo:lo + 64, r0:r1, h])
                    else:  # 64-partition window at offset 32: two halves
                        nc.scalar.copy(bias2[h * 64:h * 64 + 32, r0:r1],
                                       wT_v[lo:lo + 32, r0:r1, h])
                        nc.scalar.copy(bias2[h * 64 + 32:(h + 1) * 64, r0:r1],
                                       wT_v[lo + 32:lo + 64, r0:r1, h])

            # ---- accumulators ----
            Sacc = singles.tile([128, NP], f32)
            Cacc = singles.tile([128, NP], f32)

            # ---- main loop: one pair of anchors per tile ----
            for ar in range(NP):
                zb_ps = wpsum.tile([128, B], f32)
                nc.tensor.matmul(zb_ps, lhsT=sel_all[:, ar, :, :], rhs=z_bf)
                r = rpool.tile([128, B], bf16)
                nc.scalar.activation(
                    out=r, in_=zb_ps, func=Act.Relu,
                    bias=bias2[:, ar:ar + 1], scale=1.0,
                    accum_out=Sacc[:, ar:ar + 1])
                m = mpool.tile([128, B], bf16)
                nc.vector.tensor_scalar(
                    m, r, EPS, None, Alu.is_gt, Alu.add,
                    accum_out=Cacc[:, ar:ar + 1])

            # ---- final reduce ----
            SC = singles.tile([128, 2], f32)
            nc.vector.tensor_reduce(SC[:, 0:1], Sacc, Ax.X, Alu.add)
            nc.vector.tensor_reduce(SC[:, 1:2], Cacc, Ax.X, Alu.add)
            fin_ps = spsum.tile([2, 1], f32, tag="sA")
            nc.tensor.matmul(fin_ps, lhsT=SC, rhs=ones_c)
            fin_sb = singles.tile([2, 1], f32)
            nc.scalar.copy(fin_sb, fin_ps)
            nc.sync.dma_start(
                out=out_d.ap().rearrange("(p o) -> p o", o=1), in_=fin_sb)

    nc.compile()
    return nc


def _build_dense():
    """Dense fallback (no class-size assumption)."""
    import concourse.mybir as mybir
    import concourse.tile as tile
    from concourse import bacc
    from concourse.masks import make_identity

    f32 = mybir.dt.float32
    bf16 = mybir.dt.bfloat16
    Alu = mybir.AluOpType
    Act = mybir.ActivationFunctionType
    Ax = mybir.AxisListType

    nc = bacc.Bacc("TRN2", target_bir_lowering=False, debug=False,
                   num_devices=NCORES)

    imgT_d = nc.dram_tensor("imgT", [D, A], f32, kind="ExternalInput")
    sentT_d = nc.dram_tensor("sentT", [D, B], f32, kind="ExternalInput")
    labf_d = nc.dram_tensor("labf", [B], f32, kind="ExternalInput")
    labc_d = nc.dram_tensor("labc", [A], f32, kind="ExternalInput")
    out_d = nc.dram_tensor("out", [2], f32, kind="ExternalOutput")

    with tile.TileContext(nc) as tc:
        with ExitStack() as ctx:
            singles = ctx.enter_context(tc.tile_pool(name="singles", bufs=1))
            rpool = ctx.enter_context(tc.tile_pool(name="rpool", bufs=6))
            mpool = ctx.enter_context(tc.tile_pool(name="mpool", bufs=6))
            spsum = ctx.enter_context(
                tc.tile_pool(name="spsum", bufs=1, space="PSUM"))
            wpsum = ctx.enter_context(
                tc.tile_pool(name="wpsum", bufs=3, space="PSUM"))

            ones_r = singles.tile([1, 128], f32)
            nc.vector.memset(ones_r, 1.0)
            ones_c = singles.tile([128, 1], f32)
            nc.vector.memset(ones_c, 1.0)
            ident = singles.tile([64, 64], f32)
            make_identity(nc, ident)

            imgT = singles.tile([128, KT, A], f32)
            nc.sync.dma_start(
                out=imgT, in_=imgT_d.ap().rearrange("(t p) m -> p t m", p=128))
            sentT = singles.tile([128, KT, B], f32)
            nc.sync.dma_start(
                out=sentT, in_=sentT_d.ap().rearrange("(t p) m -> p t m", p=128))
            lab_row = singles.tile([1, B], f32)
            nc.sync.dma_start(
                out=lab_row, in_=labf_d.ap().rearrange("(o b) -> o b", o=1))
            labc_col = singles.tile([A, 1], f32)
            nc.sync.dma_start(
                out=labc_col, in_=labc_d.ap().rearrange("(a o) -> a o", o=1))

            pw_ps = spsum.tile([A, B], f32)
            for kt in range(KT):
                nc.tensor.matmul(pw_ps, lhsT=imgT[:, kt, :], rhs=sentT[:, kt, :],
                                 start=(kt == 0), stop=(kt == KT - 1))

            labB_ps = spsum.tile([A, B], f32)
            nc.tensor.matmul(labB_ps, lhsT=ones_r[:, :A], rhs=lab_row)
            eqP = singles.tile([A, B], f32)
            nc.vector.tensor_scalar(eqP, labB_ps, labc_col, None, Alu.is_equal)
            penP = singles.tile([A, B], f32)
            nc.vector.tensor_scalar(penP, eqP, 1.0, BIG, Alu.subtract, Alu.mult)
            penN = singles.tile([A, B], f32)
            nc.vector.tensor_scalar(penN, eqP, -BIG, None, Alu.mult)

            w = singles.tile([A, B], f32)
            nc.vector.tensor_scalar(w, pw_ps, MARGIN, None, Alu.add)
            nc.vector.tensor_mul(w, w, eqP)
            nc.vector.tensor_add(w, w, penP)
            negneq = singles.tile([A, B], f32)
            nc.vector.tensor_scalar(negneq, eqP, 1.0, -1.0, Alu.subtract,
                                    Alu.mult)
            z = singles.tile([A, B], f32)
            nc.vector.tensor_scalar(z, pw_ps, -1.0, None, Alu.mult)
            nc.vector.tensor_mul(z, z, negneq)
            nc.vector.tensor_add(z, z, penN)

            zTs = singles.tile([128, NT, A], f32)
            for j in range(NT):
                zt_ps = spsum.tile([128, A], f32)
                nc.tensor.transpose(zt_ps, z[:, j * 128:(j + 1) * 128], ident)
                nc.scalar.copy(zTs[:, j, :], zt_ps)

            Sacc = singles.tile([128, A * NT], f32)
            Cacc = singles.tile([128, A * NT], f32)

            for a in range(A):
                wb_ps = wpsum.tile([128, B], f32)
                nc.tensor.matmul(
                    wb_ps, lhsT=ident[:, a:a + 1].broadcast_to([A, 128]), rhs=w)
                for j in range(NT):
                    col = a * NT + j
                    r = rpool.tile([128, B], bf16)
                    nc.scalar.activation(
                        out=r, in_=wb_ps, func=Act.Relu,
                        bias=zTs[:, j, a:a + 1], scale=1.0,
                        accum_out=Sacc[:, col:col + 1])
                    m = mpool.tile([128, B], bf16)
                    nc.vector.tensor_scalar(
                        m, r, EPS, None, Alu.is_gt, Alu.add,
                        accum_out=Cacc[:, col:col + 1])

            SC = singles.tile([128, 2], f32)
            nc.vector.tensor_reduce(SC[:, 0:1], Sacc, Ax.X, Alu.add)
            nc.vector.tensor_reduce(SC[:, 1:2], Cacc, Ax.X, Alu.add)
            fin_ps = spsum.tile([2, 1], f32)
            nc.tensor.matmul(fin_ps, lhsT=SC, rhs=ones_c)
            fin_sb = singles.tile([2, 1], f32)
            nc.scalar.copy(fin_sb, fin_ps)
            nc.sync.dma_start(
                out=out_d.ap().rearrange("(p o) -> p o", o=1), in_=fin_sb)

    nc.compile()
    return nc


def _get_nc(variant):
    key = f"nc_{variant}"
    if key not in _CACHE:
        _CACHE[key] = _build_win() if variant == "win" else _build_dense()
    return _CACHE[key]


def _prep(labels, image_embeddings, sentence_embeddings):
    """Class-sort the batch; build per-core input maps."""
    labels = np.ascontiguousarray(labels)
    img = np.ascontiguousarray(image_embeddings, dtype=np.float32)
    sent = np.ascontiguousarray(sentence_embeddings, dtype=np.float32)
    counts = np.bincount(labels.astype(np.int64))
    variant = "win" if counts.max() <= MAXC_WIN else "dense"

    perm = np.argsort(labels, kind="stable")
    labs = labels[perm].astype(np.float32)
    imgT = np.ascontiguousarray(img[perm].T)    # [D, B]
    sentT = np.ascontiguousarray(sent[perm].T)  # [D, B]
    if variant == "win":
        import ml_dtypes
        imgT = imgT.astype(ml_dtypes.bfloat16)
        sentT = sentT.astype(ml_dtypes.bfloat16)

    maps = []
    for i in range(NCORES):
        c0 = i * A
        m = {
            "imgT": np.ascontiguousarray(imgT[:, c0:c0 + A]),
            "sentT": sentT,
            "labf": labs,
            "labc": np.ascontiguousarray(labs[c0:c0 + A]),
        }
        if variant == "win":
            lo, hi = c0 - 32, c0 + 96
            swin = np.zeros((D, W), sentT.dtype)
            lwin = np.full((W,), -1.0, np.float32)
            s0, s1 = max(lo, 0), min(hi, B)
            swin[:, s0 - lo:s1 - lo] = sentT[:, s0:s1]
            lwin[s0 - lo:s1 - lo] = labs[s0:s1]
            m["sentWinT"] = swin
            m["labw"] = lwin
        maps.append(m)
    return variant, maps


def run_all(labels, image_embeddings, sentence_embeddings, trace=False):
    from concourse.bass_utils import run_bass_kernel_spmd
    variant, maps = _prep(labels, image_embeddings, sentence_embeddings)
    nc = _get_nc(variant)
    res = run_bass_kernel_spmd(nc, maps, list(range(NCORES)), trace=trace)
    parts = np.stack([res.results[i]["out"] for i in range(NCORES)])
    s = float(parts[:, 0].sum())
    c = float(parts[:, 1].sum())
    loss = np.float32(s / (c + EPS))
    return np.asarray(loss, dtype=np.float32), res


def kernel(labels, image_embeddings, sentence_embeddings):
    out, _ = run_all(labels, image_embeddings, sentence_embeddings)
    return out
